# revision 2
# baseline (speedup 1.0000x reference)
"""CPC NCE loss kernel for Trainium2, 8 NeuronCores.

Sharding: the 224 independent (i,k,j) NCE combos are split 28 per core.
Per core the 28 combos form 7 "units" (one (i,k) pair restricted to 4
consecutive j positions = 256 rows) of 2 "chunks" (128 rows) each.

Per chunk (128 rows r = (j, b)):
  zh^T = Wk^T.T @ C^T   (PE, bf16, accumulated f32 in PSUM, + bias via ACT)
  raw  = zh @ Zneg      (PE; Zneg is the shared (512, 4096) negatives matrix
                         laid out n = (h*8+w)*64 + b so the NCE mask is the
                         same diagonal pattern for every 512-wide bank)
  pos  = diag(zh @ Zpos^T)  (PE 128x128 + DVE eye-masked row-sum)
  masked max + 0-clamp   (DVE tensor_tensor_reduce, additive -1e4 mask,
                          reduce initial value 0.0)
  sum exp(raw - max)     (ACT Exp with per-partition bias, fused accum)
  nce = pos_shift - log(exp(pos_shift) + sumexp)
Host sums the 8 cores' (128, 14) partial nce tiles and takes -mean.
"""

import numpy as np
import ml_dtypes

import concourse.bass as bass
import concourse.tile as tile
from concourse import mybir
from concourse.vector_clock import ScopedClock
from concourse.bass_utils import run_bass_kernel_spmd

B, D, H, W = 64, 512, 8, 8
NCORES = 8
NUNITS = 7           # units per core
NCHUNKS = 2 * NUNITS  # chunks per core
NB = 8               # 512-wide negative banks per chunk
EC = 4               # 128-wide feature chunks
BF16 = ml_dtypes.bfloat16
MASK_VAL = -10000.0

F32 = mybir.dt.float32
BF = mybir.dt.bfloat16

LAST_RESULTS = None  # BassKernelResults of the most recent run (for test.py)

_cache = {}


def _split_multi_waits(nc):
    """walrus in this container accepts at most ONE sync wait per
    instruction; hoist extra waits onto preceding same-engine NOPs."""
    k = 0
    for f in nc.m.functions:
        for bb in f.blocks:
            newlist = []
            changed = False
            for inst in bb.instructions:
                si = inst.sync_info
                if si is not None and si.on_wait and len(si.on_wait) > 1:
                    waits = list(si.on_wait)
                    for w in waits[:-1]:
                        nop = mybir.InstNoOp(name=f"I-wsplit-{k}", ins=[], outs=[])
                        k += 1
                        nop.engine = inst.engine
                        nop.sync_info = mybir.SyncInfo(on_wait=[w], on_update=[])
                        newlist.append(nop)
                    inst.sync_info = mybir.SyncInfo(
                        on_wait=[waits[-1]], on_update=list(si.on_update or [])
                    )
                    changed = True
                newlist.append(inst)
            if changed:
                bb.instructions = newlist


class _TileContext(tile.TileContext):
    """Tail drain variant that keeps <=1 sem wait per instruction."""

    def _drain_and_barrier(self, tick_clock, wait_clock):
        nc = self.nc
        probe = nc.sync.nop(nofuse=True)
        wait_clock.add_sem_waits(
            probe.ins, ScopedClock({None: tick_clock.global_clock})
        )
        si = probe.ins.sync_info
        if si is not None and si.on_wait and len(si.on_wait) > 1:
            waits = list(si.on_wait)
            probe.ins.sync_info = mybir.SyncInfo(
                on_wait=waits[:1], on_update=list(si.on_update or [])
            )
            for w in waits[1:]:
                n2 = nc.sync.nop(nofuse=True)
                n2.ins.sync_info = mybir.SyncInfo(on_wait=[w], on_update=[])
        nc.sync.drain()
        nc.all_engine_barrier()
        assert self.sems is not None
        popped = nc._tile_sem_poison_stack.pop()
        assert popped is self._sem_poison
        nc.clear_and_free_semaphores(list(self.sems.allocated().values()))
        nc.all_engine_barrier()


def _build_module():
    nc = bass.Bass("TRN2", target_bir_lowering=False, debug=False)
    ap = {}
    ap["zn"] = nc.dram_tensor("zn", [EC, 128, 4096], BF, kind="ExternalInput").ap()
    ap["wtc"] = nc.dram_tensor("wtc", [NUNITS, EC, 128, 512], BF, kind="ExternalInput").ap()
    ap["ctc"] = nc.dram_tensor("ctc", [NUNITS, EC, 128, 256], BF, kind="ExternalInput").ap()
    ap["zpc"] = nc.dram_tensor("zpc", [NCHUNKS, EC, 128, 128], BF, kind="ExternalInput").ap()
    ap["bgc"] = nc.dram_tensor("bgc", [NUNITS, 128, EC], F32, kind="ExternalInput").ap()
    ap["addm"] = nc.dram_tensor("addm", [128, 512], F32, kind="ExternalInput").ap()
    ap["eye"] = nc.dram_tensor("eye", [128, 128], F32, kind="ExternalInput").ap()
    out_ap = nc.dram_tensor("out", [128, NCHUNKS], F32, kind="ExternalOutput").ap()

    Exp = mybir.ActivationFunctionType.Exp
    Ln = mybir.ActivationFunctionType.Ln
    Ident = mybir.ActivationFunctionType.Identity
    Add = mybir.AluOpType.add
    Mult = mybir.AluOpType.mult
    Max = mybir.AluOpType.max
    Sub = mybir.AluOpType.subtract
    X = mybir.AxisListType.X

    with _TileContext(nc) as tc:
        with (
            tc.tile_pool(name="consts", bufs=1) as consts,
            tc.tile_pool(name="wpool", bufs=2) as wpool,
            tc.tile_pool(name="cpool", bufs=2) as cpool,
            tc.tile_pool(name="zhpool", bufs=2) as zhpool,
            tc.tile_pool(name="zppool", bufs=2) as zppool,
            tc.tile_pool(name="bgpool", bufs=2) as bgpool,
            tc.tile_pool(name="mpool", bufs=2) as mpool,
            tc.tile_pool(name="scr", bufs=2) as scr,
            tc.tile_pool(name="smalls", bufs=4) as smalls,
            tc.tile_pool(name="ps_raw", bufs=5, space="PSUM") as ps_raw,
            tc.tile_pool(name="ps_zh", bufs=1, space="PSUM") as ps_zh,
            tc.tile_pool(name="ps_pos", bufs=1, space="PSUM") as ps_pos,
        ):
            zn_t = consts.tile([128, EC, 4096], BF)
            for dc in range(EC):
                nc.sync.dma_start(zn_t[:, dc, :], ap["zn"][dc])
            addm_t = consts.tile([128, 512], F32)
            nc.sync.dma_start(addm_t[:], ap["addm"][:])
            eye_t = consts.tile([128, 128], F32)
            nc.sync.dma_start(eye_t[:], ap["eye"][:])
            out_t = consts.tile([128, NCHUNKS], F32)

            for u in range(NUNITS):
                wt = wpool.tile([128, EC, 512], BF)
                ct = cpool.tile([128, EC, 256], BF)
                for dc in range(EC):
                    nc.sync.dma_start(wt[:, dc, :], ap["wtc"][u, dc])
                    nc.sync.dma_start(ct[:, dc, :], ap["ctc"][u, dc])
                bg = bgpool.tile([128, EC], F32)
                nc.sync.dma_start(bg[:], ap["bgc"][u])

                # mm1: zh^T[e, r] for the unit's 256 rows
                zh_ps = ps_zh.tile([128, EC, 256], F32)
                for ec in range(EC):
                    for dc in range(EC):
                        nc.tensor.matmul(
                            zh_ps[:, ec, :],
                            wt[:, dc, ec * 128:(ec + 1) * 128],
                            ct[:, dc, :],
                            start=(dc == 0),
                            stop=(dc == EC - 1),
                        )
                zh = zhpool.tile([128, EC, 256], BF)
                for ec in range(EC):
                    nc.scalar.activation(
                        zh[:, ec, :], zh_ps[:, ec, :], Ident,
                        bias=bg[:, ec:ec + 1], scale=1.0,
                    )

                for h_ in range(2):
                    t_idx = 2 * u + h_
                    zp = zppool.tile([128, EC, 128], BF)
                    for dc in range(EC):
                        nc.sync.dma_start(zp[:, dc, :], ap["zpc"][t_idx, dc])

                    rs = slice(h_ * 128, (h_ + 1) * 128)

                    # positives: diag(zh_chunk @ Zpos^T)
                    pos_ps = ps_pos.tile([128, 128], F32)
                    for ec in range(EC):
                        nc.tensor.matmul(
                            pos_ps[:], zh[:, ec, rs], zp[:, ec, :],
                            start=(ec == 0), stop=(ec == EC - 1),
                        )

                    masked = mpool.tile([128, 4096], F32)
                    pmax = smalls.tile([128, NB], F32)
                    for nb in range(NB):
                        raw_ps = ps_raw.tile([128, 512], F32)
                        for ec in range(EC):
                            nc.tensor.matmul(
                                raw_ps[:],
                                zh[:, ec, rs],
                                zn_t[:, ec, nb * 512:(nb + 1) * 512],
                                start=(ec == 0),
                                stop=(ec == EC - 1),
                            )
                        # masked = raw + addmask ; pmax[nb] = rowmax(masked)
                        msl = masked[:, nb * 512:(nb + 1) * 512]
                        nc.vector.tensor_add(msl, raw_ps[:], addm_t[:])
                        nc.vector.reduce_max(
                            out=pmax[:, nb:nb + 1], in_=msl, axis=X
                        )

                    dsc = scr.tile([128, 128], F32)
                    pos_sb = smalls.tile([128, 1], F32)
                    nc.vector.tensor_mul(dsc[:], pos_ps[:], eye_t[:])
                    nc.vector.reduce_sum(out=pos_sb[:], in_=dsc[:], axis=X)

                    rowmax = smalls.tile([128, 1], F32)
                    nc.vector.reduce_max(out=rowmax[:], in_=pmax[:], axis=X)
                    # negbias = -max(rowmax, 0)
                    negbias = smalls.tile([128, 1], F32)
                    nc.vector.tensor_scalar(
                        out=negbias[:], in0=rowmax[:],
                        scalar1=0.0, scalar2=-1.0, op0=Max, op1=Mult,
                    )

                    sumexp = smalls.tile([128, NB], F32)
                    for nb in range(NB):
                        esc = scr.tile([128, 512], F32)
                        nc.scalar.activation(
                            esc[:], masked[:, nb * 512:(nb + 1) * 512], Exp,
                            bias=negbias[:, 0:1], scale=1.0,
                            accum_out=sumexp[:, nb:nb + 1],
                        )
                    S = smalls.tile([128, 1], F32)
                    nc.vector.reduce_sum(out=S[:], in_=sumexp[:], axis=X)
                    E = smalls.tile([128, 1], F32)
                    nc.scalar.activation(
                        E[:], pos_sb[:], Exp, bias=negbias[:, 0:1], scale=1.0
                    )
                    T = smalls.tile([128, 1], F32)
                    nc.vector.tensor_add(T[:], E[:], S[:])
                    L = smalls.tile([128, 1], F32)
                    nc.scalar.activation(L[:], T[:], Ln)
                    # nce = (pos + negbias) - L
                    nc.vector.scalar_tensor_tensor(
                        out=out_t[:, t_idx:t_idx + 1],
                        in0=pos_sb[:],
                        scalar=negbias[:, 0:1],
                        in1=L[:],
                        op0=Add,
                        op1=Sub,
                    )

            nc.sync.dma_start(out_ap[:], out_t[:])

    _split_multi_waits(nc)
    return nc


def _prep_inputs(Z, C, Wk, bk):
    """Host-side layout prep + per-core slicing."""
    ii, kk = np.triu_indices(H, 1)
    zn = np.ascontiguousarray(
        Z.transpose(1, 2, 3, 0).reshape(D, H * W * B).reshape(EC, 128, H * W * B)
    ).astype(BF16)
    WkT = np.ascontiguousarray(Wk.transpose(0, 2, 1)).astype(BF16)  # (7, d, e)
    Ctr = np.ascontiguousarray(C.transpose(2, 1, 3, 0))  # (H, D, W, B)
    Ztr = np.ascontiguousarray(Z.transpose(2, 1, 3, 0))  # (H, D, W, B)

    rr = np.arange(128)
    addm = np.where(
        (np.arange(512)[None, :] % 64) == (rr[:, None] % 64),
        np.float32(MASK_VAL), np.float32(0.0),
    ).astype(np.float32)
    eye = np.eye(128, dtype=np.float32)

    in_maps = []
    for c in range(NCORES):
        wtc = np.empty((NUNITS, EC, 128, 512), BF16)
        ctc = np.empty((NUNITS, EC, 128, 256), BF16)
        zpc = np.empty((NCHUNKS, EC, 128, 128), BF16)
        bgc = np.empty((NUNITS, 128, EC), np.float32)
        for u in range(NUNITS):
            g = NUNITS * c + u
            p = g // 2
            w0 = 4 * (g % 2)
            i_, k_ = int(ii[p]), int(kk[p])
            wtc[u] = WkT[k_ - 1].reshape(EC, 128, 512)
            ctc[u] = (
                Ctr[i_][:, w0:w0 + 4, :].reshape(D, 256).astype(BF16)
                .reshape(EC, 128, 256)
            )
            bgc[u] = bk[k_ - 1].reshape(EC, 128).T
            for h_ in range(2):
                wp0 = w0 + 2 * h_
                zpc[2 * u + h_] = (
                    Ztr[k_][:, wp0:wp0 + 2, :].reshape(D, 128).astype(BF16)
                    .reshape(EC, 128, 128)
                )
        in_maps.append({
            "zn": zn, "wtc": wtc, "ctc": ctc, "zpc": zpc, "bgc": bgc,
            "addm": addm, "eye": eye,
        })
    return in_maps


def kernel(Z, C, Wk, bk):
    global LAST_RESULTS
    Z = np.asarray(Z, np.float32)
    C = np.asarray(C, np.float32)
    Wk = np.asarray(Wk, np.float32)
    bk = np.asarray(bk, np.float32)

    if "nc" not in _cache:
        _cache["nc"] = _build_module()
    nc = _cache["nc"]

    in_maps = _prep_inputs(Z, C, Wk, bk)
    res = run_bass_kernel_spmd(nc, in_maps, core_ids=list(range(NCORES)))
    LAST_RESULTS = res
    total = np.float64(0.0)
    for c in range(NCORES):
        total += np.sum(res.results[c]["out"].astype(np.float64))
    loss = -(total / (NCORES * NCHUNKS * 128))
    return np.array(loss, dtype=np.float32)


# revision 5
# speedup vs baseline: 1.0727x; 1.0727x over previous
"""CPC NCE loss kernel for Trainium2, 8 NeuronCores.

Sharding: the 224 independent (i,k,j) NCE combos are split 28 per core.
Per core the 28 combos form 7 "units" (one (i,k) pair restricted to 4
consecutive j positions = 256 rows) of 2 "chunks" (128 rows) each.

Per chunk (128 rows r = (j, b)):
  zh^T = Wk^T.T @ C^T   (PE, bf16, f32 PSUM accum, + bias via ACT cast)
  raw  = zh @ Zneg      (PE; Zneg is the shared (512, 4096) negatives
                         matrix laid out n = (h*8+w)*64 + b so the NCE
                         self-batch mask is the same diagonal pattern
                         for every row block)
  pos  = diag(zh @ Zpos^T)  (PE 128x128 + DVE eye-mask + row-sum)
  masked = raw + addmask    (DVE, -1e4 at the masked positions)
  S = sum exp(masked - M)   (single 4096-wide ACT Exp with fused accum;
                             M is a constant shift - the log-sum-exp is
                             shift invariant, scores are ~[-56, 56])
  nce = (pos - M) - log(exp(pos - M) + S)
Host sums the 8 cores' (128, 14) partial nce tiles and takes -mean.
"""

import numpy as np
import ml_dtypes

import concourse.bass as bass
import concourse.tile as tile
from concourse import mybir
from concourse.vector_clock import ScopedClock
from concourse.bass_utils import run_bass_kernel_spmd

B, D, H, W = 64, 512, 8, 8
NCORES = 8
NUNITS = 7            # units per core
NCHUNKS = 2 * NUNITS  # chunks per core
NB = 8                # 512-wide negative banks per chunk
NG = 4                # 1024-wide (2-bank) PSUM groups per chunk
EC = 4                # 128-wide feature chunks
BF16 = ml_dtypes.bfloat16
MASK_VAL = -10000.0
M_SHIFT = 45.0

F32 = mybir.dt.float32
BF = mybir.dt.bfloat16

LAST_RESULTS = None  # BassKernelResults of the most recent run (for test.py)

_cache = {}


def _split_multi_waits(nc):
    """walrus in this container accepts at most ONE sync wait per
    instruction; hoist extra waits onto preceding same-engine NOPs."""
    k = 0
    for f in nc.m.functions:
        for bb in f.blocks:
            newlist = []
            changed = False
            for inst in bb.instructions:
                si = inst.sync_info
                if si is not None and si.on_wait and len(si.on_wait) > 1:
                    waits = list(si.on_wait)
                    for w in waits[:-1]:
                        nop = mybir.InstNoOp(name=f"I-wsplit-{k}", ins=[], outs=[])
                        k += 1
                        nop.engine = inst.engine
                        nop.sync_info = mybir.SyncInfo(on_wait=[w], on_update=[])
                        newlist.append(nop)
                    inst.sync_info = mybir.SyncInfo(
                        on_wait=[waits[-1]], on_update=list(si.on_update or [])
                    )
                    changed = True
                newlist.append(inst)
            if changed:
                bb.instructions = newlist


class _TileContext(tile.TileContext):
    """Tail drain variant that keeps <=1 sem wait per instruction."""

    def _drain_and_barrier(self, tick_clock, wait_clock):
        nc = self.nc
        probe = nc.sync.nop(nofuse=True)
        wait_clock.add_sem_waits(
            probe.ins, ScopedClock({None: tick_clock.global_clock})
        )
        si = probe.ins.sync_info
        if si is not None and si.on_wait and len(si.on_wait) > 1:
            waits = list(si.on_wait)
            probe.ins.sync_info = mybir.SyncInfo(
                on_wait=waits[:1], on_update=list(si.on_update or [])
            )
            for w in waits[1:]:
                n2 = nc.sync.nop(nofuse=True)
                n2.ins.sync_info = mybir.SyncInfo(on_wait=[w], on_update=[])
        nc.sync.drain()
        nc.all_engine_barrier()
        assert self.sems is not None
        popped = nc._tile_sem_poison_stack.pop()
        assert popped is self._sem_poison
        nc.clear_and_free_semaphores(list(self.sems.allocated().values()))
        nc.all_engine_barrier()


def _build_module():
    nc = bass.Bass("TRN2", target_bir_lowering=False, debug=False)
    ap = {}
    ap["zn"] = nc.dram_tensor("zn", [128, EC, 4096], BF, kind="ExternalInput").ap()
    ap["wtc"] = nc.dram_tensor("wtc", [NUNITS, 128, EC, 512], BF, kind="ExternalInput").ap()
    ap["ctc"] = nc.dram_tensor("ctc", [NUNITS, 128, EC, 256], BF, kind="ExternalInput").ap()
    ap["zpc"] = nc.dram_tensor("zpc", [NCHUNKS, 128, EC, 128], BF, kind="ExternalInput").ap()
    ap["bgc"] = nc.dram_tensor("bgc", [NUNITS, 128, EC], F32, kind="ExternalInput").ap()
    ap["addm"] = nc.dram_tensor("addm", [128, 1024], F32, kind="ExternalInput").ap()
    ap["eye"] = nc.dram_tensor("eye", [128, 128], F32, kind="ExternalInput").ap()
    out_ap = nc.dram_tensor("out", [128, NCHUNKS], F32, kind="ExternalOutput").ap()

    Exp = mybir.ActivationFunctionType.Exp
    Ln = mybir.ActivationFunctionType.Ln
    Ident = mybir.ActivationFunctionType.Identity
    Add = mybir.AluOpType.add
    Sub = mybir.AluOpType.subtract
    X = mybir.AxisListType.X

    with _TileContext(nc) as tc:
        with (
            tc.tile_pool(name="consts", bufs=1) as consts,
            tc.tile_pool(name="wpool", bufs=2) as wpool,
            tc.tile_pool(name="cpool", bufs=2) as cpool,
            tc.tile_pool(name="zhpool", bufs=2) as zhpool,
            tc.tile_pool(name="zppool", bufs=2) as zppool,
            tc.tile_pool(name="bgpool", bufs=2) as bgpool,
            tc.tile_pool(name="mpool", bufs=2) as mpool,
            tc.tile_pool(name="scr", bufs=2) as scr,
            tc.tile_pool(name="smalls", bufs=4) as smalls,
            tc.tile_pool(name="ps_raw", bufs=2, space="PSUM") as ps_raw,
            tc.tile_pool(name="ps_zh", bufs=1, space="PSUM") as ps_zh,
            tc.tile_pool(name="ps_pos", bufs=2, space="PSUM") as ps_pos,
        ):
            zn_t = consts.tile([128, EC, 4096], BF)
            nc.sync.dma_start(zn_t[:], ap["zn"][:])
            addm_t = consts.tile([128, 1024], F32)
            nc.sync.dma_start(addm_t[:], ap["addm"][:])
            eye_t = consts.tile([128, 128], F32)
            nc.sync.dma_start(eye_t[:], ap["eye"][:])
            out_t = consts.tile([128, NCHUNKS], F32)
            negM = consts.tile([128, 1], F32)
            nc.vector.memset(negM[:], -M_SHIFT)

            for u in range(NUNITS):
                wt = wpool.tile([128, EC, 512], BF)
                nc.sync.dma_start(wt[:], ap["wtc"][u])
                ct = cpool.tile([128, EC, 256], BF)
                nc.sync.dma_start(ct[:], ap["ctc"][u])
                bg = bgpool.tile([128, EC], F32)
                nc.sync.dma_start(bg[:], ap["bgc"][u])

                # mm1: zh^T[e, r] for the unit's 256 rows
                zh_ps = ps_zh.tile([128, EC, 256], F32)
                for ec in range(EC):
                    for dc in range(EC):
                        nc.tensor.matmul(
                            zh_ps[:, ec, :],
                            wt[:, dc, ec * 128:(ec + 1) * 128],
                            ct[:, dc, :],
                            start=(dc == 0),
                            stop=(dc == EC - 1),
                        )
                zh = zhpool.tile([128, EC, 256], BF)
                for ec in range(EC):
                    nc.scalar.activation(
                        zh[:, ec, :], zh_ps[:, ec, :], Ident,
                        bias=bg[:, ec:ec + 1], scale=1.0,
                    )

                for h_ in range(2):
                    t_idx = 2 * u + h_
                    zp = zppool.tile([128, EC, 128], BF)
                    nc.sync.dma_start(zp[:], ap["zpc"][t_idx])

                    rs = slice(h_ * 128, (h_ + 1) * 128)

                    # positives: diag(zh_chunk @ Zpos^T)
                    pos_ps = ps_pos.tile([128, 128], F32)
                    for ec in range(EC):
                        nc.tensor.matmul(
                            pos_ps[:], zh[:, ec, rs], zp[:, ec, :],
                            start=(ec == 0), stop=(ec == EC - 1),
                        )

                    masked = mpool.tile([128, 4096], F32)
                    for grp in range(NG):
                        raw_ps = ps_raw.tile([128, 1024], F32)
                        for q in range(2):
                            nb = 2 * grp + q
                            for ec in range(EC):
                                nc.tensor.matmul(
                                    raw_ps[:, q * 512:(q + 1) * 512],
                                    zh[:, ec, rs],
                                    zn_t[:, ec, nb * 512:(nb + 1) * 512],
                                    start=(ec == 0),
                                    stop=(ec == EC - 1),
                                )
                        # masked = raw + addmask (one 1024-wide DVE op)
                        nc.vector.tensor_add(
                            masked[:, grp * 1024:(grp + 1) * 1024],
                            raw_ps[:], addm_t[:],
                        )

                    dsc = scr.tile([128, 128], F32)
                    pos_sb = smalls.tile([128, 1], F32)
                    nc.vector.tensor_mul(dsc[:], pos_ps[:], eye_t[:])
                    nc.vector.reduce_sum(out=pos_sb[:], in_=dsc[:], axis=X)

                    # S = sum exp(masked - M) in one fused ACT pass
                    esc = scr.tile([128, 4096], F32)
                    S = smalls.tile([128, 1], F32)
                    nc.scalar.activation(
                        esc[:], masked[:], Exp,
                        bias=negM[:, 0:1], scale=1.0, accum_out=S[:],
                    )
                    E = smalls.tile([128, 1], F32)
                    nc.scalar.activation(E[:], pos_sb[:], Exp, bias=negM[:, 0:1])
                    T = smalls.tile([128, 1], F32)
                    nc.vector.tensor_add(T[:], E[:], S[:])
                    L = smalls.tile([128, 1], F32)
                    nc.scalar.activation(L[:], T[:], Ln)
                    # nce = (pos - M) - L
                    nc.vector.scalar_tensor_tensor(
                        out=out_t[:, t_idx:t_idx + 1],
                        in0=pos_sb[:],
                        scalar=-M_SHIFT,
                        in1=L[:],
                        op0=Add,
                        op1=Sub,
                    )

            nc.sync.dma_start(out_ap[:], out_t[:])

    _split_multi_waits(nc)
    return nc


def _prep_inputs(Z, C, Wk, bk):
    """Host-side layout prep + per-core slicing (partition-major so every
    SBUF tile loads with a single contiguous DMA)."""
    ii, kk = np.triu_indices(H, 1)
    zn = (
        Z.transpose(1, 2, 3, 0).reshape(EC, 128, H * W * B)
        .transpose(1, 0, 2)
    )
    zn = np.ascontiguousarray(zn).astype(BF16)  # (128, 4, 4096)
    WkT = Wk.transpose(0, 2, 1).reshape(7, EC, 128, 512).transpose(0, 2, 1, 3)
    WkT = np.ascontiguousarray(WkT).astype(BF16)  # (7, 128, 4, 512)
    Ctr = np.ascontiguousarray(C.transpose(2, 1, 3, 0))  # (H, D, W, B)
    Ztr = np.ascontiguousarray(Z.transpose(2, 1, 3, 0))  # (H, D, W, B)

    rr = np.arange(128)
    addm = np.where(
        (np.arange(1024)[None, :] % 64) == (rr[:, None] % 64),
        np.float32(MASK_VAL), np.float32(0.0),
    ).astype(np.float32)
    eye = np.eye(128, dtype=np.float32)

    in_maps = []
    for c in range(NCORES):
        wtc = np.empty((NUNITS, 128, EC, 512), BF16)
        ctc = np.empty((NUNITS, 128, EC, 256), BF16)
        zpc = np.empty((NCHUNKS, 128, EC, 128), BF16)
        bgc = np.empty((NUNITS, 128, EC), np.float32)
        for u in range(NUNITS):
            g = NUNITS * c + u
            p = g // 2
            w0 = 4 * (g % 2)
            i_, k_ = int(ii[p]), int(kk[p])
            wtc[u] = WkT[k_ - 1]
            ctc[u] = (
                Ctr[i_][:, w0:w0 + 4, :].reshape(EC, 128, 256)
                .transpose(1, 0, 2).astype(BF16)
            )
            bgc[u] = bk[k_ - 1].reshape(EC, 128).T
            for h_ in range(2):
                wp0 = w0 + 2 * h_
                zpc[2 * u + h_] = (
                    Ztr[k_][:, wp0:wp0 + 2, :].reshape(EC, 128, 128)
                    .transpose(1, 0, 2).astype(BF16)
                )
        in_maps.append({
            "zn": zn, "wtc": wtc, "ctc": ctc, "zpc": zpc, "bgc": bgc,
            "addm": addm, "eye": eye,
        })
    return in_maps


def kernel(Z, C, Wk, bk):
    global LAST_RESULTS
    Z = np.asarray(Z, np.float32)
    C = np.asarray(C, np.float32)
    Wk = np.asarray(Wk, np.float32)
    bk = np.asarray(bk, np.float32)

    if "nc" not in _cache:
        _cache["nc"] = _build_module()
    nc = _cache["nc"]

    in_maps = _prep_inputs(Z, C, Wk, bk)
    res = run_bass_kernel_spmd(nc, in_maps, core_ids=list(range(NCORES)))
    LAST_RESULTS = res
    total = np.float64(0.0)
    for c in range(NCORES):
        total += np.sum(res.results[c]["out"].astype(np.float64))
    loss = -(total / (NCORES * NCHUNKS * 128))
    return np.array(loss, dtype=np.float32)


# revision 10
# speedup vs baseline: 1.4096x; 1.3141x over previous
"""CPC NCE loss kernel for Trainium2, 8 NeuronCores.

Sharding: the 224 independent (i,k,j) NCE combos are split 28 per core.
Per core the 28 combos form 7 "units" (one (i,k) pair restricted to 4
consecutive j positions = 256 rows) of 2 "chunks" (128 rows) each.

Per chunk (128 rows r = (j, b)):
  zh^T = Wk^T.T @ C^T   (PE, bf16, f32 PSUM accum, + bias via ACT cast)
  raw  = zh @ Zneg      (PE; Zneg is the shared (512, 4096) negatives
                         matrix laid out n = (h*8+w)*64 + b so the NCE
                         self-batch mask is the same diagonal pattern
                         for every row block)
  pos  = diag(zh @ Zpos^T)  (PE 128x128 + DVE eye-mask + row-sum)
  masked = raw + addmask    (DVE, -1e4 at the masked positions)
  S = sum exp(masked - M)   (single 4096-wide ACT Exp with fused accum;
                             M is a constant shift - the log-sum-exp is
                             shift invariant, scores are ~[-56, 56])
  nce = (pos - M) - log(exp(pos - M) + S)
Host sums the 8 cores' (128, 14) partial nce tiles and takes -mean.
"""

import numpy as np
import ml_dtypes

import concourse.bass as bass
import concourse.tile as tile
from concourse import mybir
from concourse.vector_clock import ScopedClock
from concourse.bass_utils import run_bass_kernel_spmd

B, D, H, W = 64, 512, 8, 8
NCORES = 8
NUNITS = 7            # units per core
NCHUNKS = 2 * NUNITS  # chunks per core
NB = 8                # 512-wide negative banks per chunk
NG = 4                # 1024-wide (2-bank) PSUM groups per chunk
EC = 4                # 128-wide feature chunks
BF16 = ml_dtypes.bfloat16
MASK_VAL = -10000.0
M_SHIFT = 45.0

F32 = mybir.dt.float32
BF = mybir.dt.bfloat16

LAST_RESULTS = None  # BassKernelResults of the most recent run (for test.py)

_cache = {}


def _split_multi_waits(nc):
    """walrus in this container accepts at most ONE sync wait per
    instruction; hoist extra waits onto preceding same-engine NOPs."""
    k = 0
    for f in nc.m.functions:
        for bb in f.blocks:
            newlist = []
            changed = False
            for inst in bb.instructions:
                si = inst.sync_info
                if si is not None and si.on_wait and len(si.on_wait) > 1:
                    waits = list(si.on_wait)
                    for w in waits[:-1]:
                        nop = mybir.InstNoOp(name=f"I-wsplit-{k}", ins=[], outs=[])
                        k += 1
                        nop.engine = inst.engine
                        nop.sync_info = mybir.SyncInfo(on_wait=[w], on_update=[])
                        newlist.append(nop)
                    inst.sync_info = mybir.SyncInfo(
                        on_wait=[waits[-1]], on_update=list(si.on_update or [])
                    )
                    changed = True
                newlist.append(inst)
            if changed:
                bb.instructions = newlist


class _TileContext(tile.TileContext):
    """Tail drain variant that keeps <=1 sem wait per instruction."""

    def _drain_and_barrier(self, tick_clock, wait_clock):
        nc = self.nc
        probe = nc.sync.nop(nofuse=True)
        wait_clock.add_sem_waits(
            probe.ins, ScopedClock({None: tick_clock.global_clock})
        )
        si = probe.ins.sync_info
        if si is not None and si.on_wait and len(si.on_wait) > 1:
            waits = list(si.on_wait)
            probe.ins.sync_info = mybir.SyncInfo(
                on_wait=waits[:1], on_update=list(si.on_update or [])
            )
            for w in waits[1:]:
                n2 = nc.sync.nop(nofuse=True)
                n2.ins.sync_info = mybir.SyncInfo(on_wait=[w], on_update=[])
        nc.sync.drain()
        nc.all_engine_barrier()
        assert self.sems is not None
        popped = nc._tile_sem_poison_stack.pop()
        assert popped is self._sem_poison
        nc.clear_and_free_semaphores(list(self.sems.allocated().values()))
        nc.all_engine_barrier()


def _build_module():
    nc = bass.Bass("TRN2", target_bir_lowering=False, debug=False)
    ap = {}
    ap["zn"] = nc.dram_tensor("zn", [NG, 128, EC, 1024], BF, kind="ExternalInput").ap()
    ap["wtc"] = nc.dram_tensor("wtc", [NUNITS, 128, EC, 512], BF, kind="ExternalInput").ap()
    ap["ctc"] = nc.dram_tensor("ctc", [NUNITS, 128, EC, 256], BF, kind="ExternalInput").ap()
    ap["zpc"] = nc.dram_tensor("zpc", [NCHUNKS, 128, EC, 128], BF, kind="ExternalInput").ap()
    ap["bgc"] = nc.dram_tensor("bgc", [NUNITS, 128, EC], F32, kind="ExternalInput").ap()
    ap["addm"] = nc.dram_tensor("addm", [128, 1024], F32, kind="ExternalInput").ap()
    ap["eye"] = nc.dram_tensor("eye", [128, 128], F32, kind="ExternalInput").ap()
    out_ap = nc.dram_tensor("out", [128, NCHUNKS], F32, kind="ExternalOutput").ap()

    Exp = mybir.ActivationFunctionType.Exp
    Ln = mybir.ActivationFunctionType.Ln
    Ident = mybir.ActivationFunctionType.Identity
    Add = mybir.AluOpType.add
    Sub = mybir.AluOpType.subtract
    X = mybir.AxisListType.X

    with _TileContext(nc) as tc:
        with (
            tc.tile_pool(name="consts", bufs=1) as consts,
            tc.tile_pool(name="wpool", bufs=3) as wpool,
            tc.tile_pool(name="cpool", bufs=3) as cpool,
            tc.tile_pool(name="zhpool", bufs=2) as zhpool,
            tc.tile_pool(name="zppool", bufs=3) as zppool,
            tc.tile_pool(name="bgpool", bufs=3) as bgpool,
            tc.tile_pool(name="mpool", bufs=2) as mpool,
            tc.tile_pool(name="scr", bufs=2) as scr,
            tc.tile_pool(name="smalls", bufs=4) as smalls,
            tc.tile_pool(name="ps_raw", bufs=2, space="PSUM") as ps_raw,
            tc.tile_pool(name="ps_zh", bufs=1, space="PSUM") as ps_zh,
            tc.tile_pool(name="ps_pos", bufs=2, space="PSUM") as ps_pos,
        ):
            def load_unit(u):
                wt = wpool.tile([128, EC, 512], BF)
                nc.sync.dma_start(wt[:], ap["wtc"][u])
                ct = cpool.tile([128, EC, 256], BF)
                nc.sync.dma_start(ct[:], ap["ctc"][u])
                bg = bgpool.tile([128, EC], F32)
                nc.sync.dma_start(bg[:], ap["bgc"][u])
                return wt, ct, bg

            def mm1(wt, ct, bg):
                """zh^T[e, r] for a unit's 256 rows, bias-added, cast bf16."""
                zh_ps = ps_zh.tile([128, EC, 256], F32)
                for ec in range(EC):
                    for dc in range(EC):
                        nc.tensor.matmul(
                            zh_ps[:, ec, :],
                            wt[:, dc, ec * 128:(ec + 1) * 128],
                            ct[:, dc, :],
                            start=(dc == 0),
                            stop=(dc == EC - 1),
                        )
                zh = zhpool.tile([128, EC, 256], BF)
                for ec in range(EC):
                    nc.scalar.activation(
                        zh[:, ec, :], zh_ps[:, ec, :], Ident,
                        bias=bg[:, ec:ec + 1], scale=1.0,
                    )
                return zh

            u0 = load_unit(0)
            addm_t = consts.tile([128, 1024], F32)
            nc.sync.dma_start(addm_t[:], ap["addm"][:])
            eye_t = consts.tile([128, 128], F32)
            nc.sync.dma_start(eye_t[:], ap["eye"][:])
            u1 = load_unit(1)
            zn_t = consts.tile([128, NG, EC, 1024], BF)
            for g in range(NG):
                nc.sync.dma_start(zn_t[:, g], ap["zn"][g])
            out_t = consts.tile([128, NCHUNKS], F32)
            negM = consts.tile([128, 1], F32)
            nc.vector.memset(negM[:], -M_SHIFT)

            pending = u1
            zh = mm1(*u0)
            for u in range(NUNITS):
                # pipeline: next unit's linear layer first so its zh is
                # ready (and ACT casts aren't queued behind this unit's
                # big exp ops)
                zh_next = None
                if u + 1 < NUNITS:
                    zh_next = mm1(*pending)
                    if u + 2 < NUNITS:
                        pending = load_unit(u + 2)

                for h_ in range(2):
                    t_idx = 2 * u + h_
                    zp = zppool.tile([128, EC, 128], BF)
                    nc.sync.dma_start(zp[:], ap["zpc"][t_idx])

                    rs = slice(h_ * 128, (h_ + 1) * 128)

                    # positives: diag(zh_chunk @ Zpos^T)
                    pos_ps = ps_pos.tile([128, 128], F32)
                    for ec in range(EC):
                        nc.tensor.matmul(
                            pos_ps[:], zh[:, ec, rs], zp[:, ec, :],
                            start=(ec == 0), stop=(ec == EC - 1),
                        )

                    masked = mpool.tile([128, 4096], F32)
                    for grp in range(NG):
                        raw_ps = ps_raw.tile([128, 1024], F32)
                        for q in range(2):
                            for ec in range(EC):
                                nc.tensor.matmul(
                                    raw_ps[:, q * 512:(q + 1) * 512],
                                    zh[:, ec, rs],
                                    zn_t[:, grp, ec, q * 512:(q + 1) * 512],
                                    start=(ec == 0),
                                    stop=(ec == EC - 1),
                                )
                        # masked = raw + addmask (one 1024-wide DVE op)
                        nc.vector.tensor_add(
                            masked[:, grp * 1024:(grp + 1) * 1024],
                            raw_ps[:], addm_t[:],
                        )

                    dsc = scr.tile([128, 128], F32)
                    pos_sb = smalls.tile([128, 1], F32)
                    nc.vector.tensor_mul(dsc[:], pos_ps[:], eye_t[:])
                    nc.vector.reduce_sum(out=pos_sb[:], in_=dsc[:], axis=X)

                    # S = sum exp(masked - M) in one fused ACT pass
                    esc = scr.tile([128, 4096], F32)
                    S = smalls.tile([128, 1], F32)
                    nc.scalar.activation(
                        esc[:], masked[:], Exp,
                        bias=negM[:, 0:1], scale=1.0, accum_out=S[:],
                    )
                    E = smalls.tile([128, 1], F32)
                    nc.scalar.activation(E[:], pos_sb[:], Exp, bias=negM[:, 0:1])
                    T = smalls.tile([128, 1], F32)
                    nc.vector.tensor_add(T[:], E[:], S[:])
                    L = smalls.tile([128, 1], F32)
                    nc.scalar.activation(L[:], T[:], Ln)
                    # nce = (pos - M) - L
                    nc.vector.scalar_tensor_tensor(
                        out=out_t[:, t_idx:t_idx + 1],
                        in0=pos_sb[:],
                        scalar=-M_SHIFT,
                        in1=L[:],
                        op0=Add,
                        op1=Sub,
                    )

                zh = zh_next

            nc.sync.dma_start(out_ap[:], out_t[:])

    _split_multi_waits(nc)
    return nc


def _prep_inputs(Z, C, Wk, bk):
    """Host-side layout prep + per-core slicing (partition-major so every
    SBUF tile loads with a single contiguous DMA)."""
    ii, kk = np.triu_indices(H, 1)
    # (NG, 128, EC, 1024): negatives matrix split into 4 column quarters
    zn = (
        Z.transpose(1, 2, 3, 0).reshape(EC, 128, NG, 1024)
        .transpose(2, 1, 0, 3)
    )
    zn = np.ascontiguousarray(zn).astype(BF16)
    WkT = Wk.transpose(0, 2, 1).reshape(7, EC, 128, 512).transpose(0, 2, 1, 3)
    WkT = np.ascontiguousarray(WkT).astype(BF16)  # (7, 128, 4, 512)
    Ctr = np.ascontiguousarray(C.transpose(2, 1, 3, 0))  # (H, D, W, B)
    Ztr = np.ascontiguousarray(Z.transpose(2, 1, 3, 0))  # (H, D, W, B)

    rr = np.arange(128)
    addm = np.where(
        (np.arange(1024)[None, :] % 64) == (rr[:, None] % 64),
        np.float32(MASK_VAL), np.float32(0.0),
    ).astype(np.float32)
    eye = np.eye(128, dtype=np.float32)

    in_maps = []
    for c in range(NCORES):
        wtc = np.empty((NUNITS, 128, EC, 512), BF16)
        ctc = np.empty((NUNITS, 128, EC, 256), BF16)
        zpc = np.empty((NCHUNKS, 128, EC, 128), BF16)
        bgc = np.empty((NUNITS, 128, EC), np.float32)
        for u in range(NUNITS):
            g = NUNITS * c + u
            p = g // 2
            w0 = 4 * (g % 2)
            i_, k_ = int(ii[p]), int(kk[p])
            wtc[u] = WkT[k_ - 1]
            ctc[u] = (
                Ctr[i_][:, w0:w0 + 4, :].reshape(EC, 128, 256)
                .transpose(1, 0, 2).astype(BF16)
            )
            bgc[u] = bk[k_ - 1].reshape(EC, 128).T
            for h_ in range(2):
                wp0 = w0 + 2 * h_
                zpc[2 * u + h_] = (
                    Ztr[k_][:, wp0:wp0 + 2, :].reshape(EC, 128, 128)
                    .transpose(1, 0, 2).astype(BF16)
                )
        in_maps.append({
            "zn": zn, "wtc": wtc, "ctc": ctc, "zpc": zpc, "bgc": bgc,
            "addm": addm, "eye": eye,
        })
    return in_maps


def kernel(Z, C, Wk, bk):
    global LAST_RESULTS
    Z = np.asarray(Z, np.float32)
    C = np.asarray(C, np.float32)
    Wk = np.asarray(Wk, np.float32)
    bk = np.asarray(bk, np.float32)

    if "nc" not in _cache:
        _cache["nc"] = _build_module()
    nc = _cache["nc"]

    in_maps = _prep_inputs(Z, C, Wk, bk)
    res = run_bass_kernel_spmd(nc, in_maps, core_ids=list(range(NCORES)))
    LAST_RESULTS = res
    total = np.float64(0.0)
    for c in range(NCORES):
        total += np.sum(res.results[c]["out"].astype(np.float64))
    loss = -(total / (NCORES * NCHUNKS * 128))
    return np.array(loss, dtype=np.float32)


# revision 14
# speedup vs baseline: 1.4297x; 1.0143x over previous
"""CPC NCE loss kernel for Trainium2, 8 NeuronCores.

Sharding: the 224 independent (i,k,j) NCE combos are split 28 per core.
Per core the 28 combos form 7 "units" (one (i,k) pair restricted to 4
consecutive j positions = 256 rows) of 2 "chunks" (128 rows) each.

Per chunk (128 rows r = (j, b)):
  zh^T = Wk^T.T @ C^T   (PE, bf16, f32 PSUM accum, + bias via ACT cast)
  raw  = zh @ Zneg      (PE; Zneg is the shared (512, 4096) negatives
                         matrix laid out n = (h*8+w)*64 + b so the NCE
                         self-batch mask is the same diagonal pattern
                         for every row block)
  pos  = diag(zh @ Zpos^T)  (PE 128x128 + DVE eye-mask + row-sum)
  masked = raw + addmask    (DVE, -1e4 at the masked positions)
  S = sum exp(masked - M)   (single 4096-wide ACT Exp with fused accum;
                             M is a constant shift - the log-sum-exp is
                             shift invariant, scores are ~[-56, 56])
  nce = (pos - M) - log(exp(pos - M) + S)
Host sums the 8 cores' (128, 14) partial nce tiles and takes -mean.
"""

import numpy as np
import ml_dtypes

import concourse.bass as bass
import concourse.tile as tile
from concourse import mybir
from concourse.vector_clock import ScopedClock
from concourse.bass_utils import run_bass_kernel_spmd

B, D, H, W = 64, 512, 8, 8
NCORES = 8
NUNITS = 7            # units per core
NCHUNKS = 2 * NUNITS  # chunks per core
NB = 8                # 512-wide negative banks per chunk
NG = 4                # 1024-wide (2-bank) PSUM groups per chunk
EC = 4                # 128-wide feature chunks
BF16 = ml_dtypes.bfloat16
MASK_VAL = -10000.0
M_SHIFT = 45.0

F32 = mybir.dt.float32
BF = mybir.dt.bfloat16

LAST_RESULTS = None  # BassKernelResults of the most recent run (for test.py)

_cache = {}


def _split_multi_waits(nc):
    """walrus in this container accepts at most ONE sync wait per
    instruction; hoist extra waits onto preceding same-engine NOPs."""
    k = 0
    for f in nc.m.functions:
        for bb in f.blocks:
            newlist = []
            changed = False
            for inst in bb.instructions:
                si = inst.sync_info
                if si is not None and si.on_wait and len(si.on_wait) > 1:
                    waits = list(si.on_wait)
                    for w in waits[:-1]:
                        nop = mybir.InstNoOp(name=f"I-wsplit-{k}", ins=[], outs=[])
                        k += 1
                        nop.engine = inst.engine
                        nop.sync_info = mybir.SyncInfo(on_wait=[w], on_update=[])
                        newlist.append(nop)
                    inst.sync_info = mybir.SyncInfo(
                        on_wait=[waits[-1]], on_update=list(si.on_update or [])
                    )
                    changed = True
                newlist.append(inst)
            if changed:
                bb.instructions = newlist


class _TileContext(tile.TileContext):
    """Tail drain variant that keeps <=1 sem wait per instruction."""

    def _drain_and_barrier(self, tick_clock, wait_clock):
        nc = self.nc
        probe = nc.sync.nop(nofuse=True)
        wait_clock.add_sem_waits(
            probe.ins, ScopedClock({None: tick_clock.global_clock})
        )
        si = probe.ins.sync_info
        if si is not None and si.on_wait and len(si.on_wait) > 1:
            waits = list(si.on_wait)
            probe.ins.sync_info = mybir.SyncInfo(
                on_wait=waits[:1], on_update=list(si.on_update or [])
            )
            for w in waits[1:]:
                n2 = nc.sync.nop(nofuse=True)
                n2.ins.sync_info = mybir.SyncInfo(on_wait=[w], on_update=[])
        nc.sync.drain()
        nc.all_engine_barrier()
        assert self.sems is not None
        popped = nc._tile_sem_poison_stack.pop()
        assert popped is self._sem_poison
        nc.clear_and_free_semaphores(list(self.sems.allocated().values()))
        nc.all_engine_barrier()


def _build_module():
    nc = bass.Bass("TRN2", target_bir_lowering=False, debug=False)
    ap = {}
    ap["zn"] = nc.dram_tensor("zn", [NG, 128, EC, 1024], BF, kind="ExternalInput").ap()
    ap["wtc"] = nc.dram_tensor("wtc", [NUNITS, 128, EC, 512], BF, kind="ExternalInput").ap()
    ap["ctc"] = nc.dram_tensor("ctc", [NUNITS, 128, EC, 256], BF, kind="ExternalInput").ap()
    ap["zpc"] = nc.dram_tensor("zpc", [NCHUNKS, 128, EC, 128], BF, kind="ExternalInput").ap()
    ap["bgc"] = nc.dram_tensor("bgc", [NUNITS, 128, EC], F32, kind="ExternalInput").ap()
    ap["addm"] = nc.dram_tensor("addm", [128, 1024], F32, kind="ExternalInput").ap()
    ap["eye"] = nc.dram_tensor("eye", [128, 128], F32, kind="ExternalInput").ap()
    out_ap = nc.dram_tensor("out", [128, NCHUNKS], F32, kind="ExternalOutput").ap()

    Exp = mybir.ActivationFunctionType.Exp
    Ln = mybir.ActivationFunctionType.Ln
    Ident = mybir.ActivationFunctionType.Identity
    Add = mybir.AluOpType.add
    Sub = mybir.AluOpType.subtract
    X = mybir.AxisListType.X

    with _TileContext(nc) as tc:
        with (
            tc.tile_pool(name="consts", bufs=1) as consts,
            tc.tile_pool(name="wpool", bufs=3) as wpool,
            tc.tile_pool(name="cpool", bufs=3) as cpool,
            tc.tile_pool(name="zhpool", bufs=2) as zhpool,
            tc.tile_pool(name="zppool", bufs=4) as zppool,
            tc.tile_pool(name="bgpool", bufs=3) as bgpool,
            tc.tile_pool(name="mpool", bufs=2) as mpool,
            tc.tile_pool(name="scr", bufs=2) as scr,
            tc.tile_pool(name="smalls", bufs=4) as smalls,
            tc.tile_pool(name="ps_raw", bufs=2, space="PSUM") as ps_raw,
            tc.tile_pool(name="ps_zh", bufs=1, space="PSUM") as ps_zh,
            tc.tile_pool(name="ps_pos", bufs=2, space="PSUM") as ps_pos,
        ):
            def load_unit(u):
                wt = wpool.tile([128, EC, 512], BF)
                nc.sync.dma_start(wt[:], ap["wtc"][u])
                ct = cpool.tile([128, EC, 256], BF)
                nc.sync.dma_start(ct[:], ap["ctc"][u])
                bg = bgpool.tile([128, EC], F32)
                nc.sync.dma_start(bg[:], ap["bgc"][u])
                return wt, ct, bg

            def mm1(wt, ct, bg):
                """zh^T[e, r] for a unit's 256 rows, bias-added, cast bf16."""
                zh_ps = ps_zh.tile([128, EC, 256], F32)
                for ec in range(EC):
                    for dc in range(EC):
                        nc.tensor.matmul(
                            zh_ps[:, ec, :],
                            wt[:, dc, ec * 128:(ec + 1) * 128],
                            ct[:, dc, :],
                            start=(dc == 0),
                            stop=(dc == EC - 1),
                        )
                zh = zhpool.tile([128, EC, 256], BF)
                for ec in range(EC):
                    nc.scalar.activation(
                        zh[:, ec, :], zh_ps[:, ec, :], Ident,
                        bias=bg[:, ec:ec + 1], scale=1.0,
                    )
                return zh

            def load_zp(t_idx):
                zp = zppool.tile([128, EC, 128], BF)
                nc.sync.dma_start(zp[:], ap["zpc"][t_idx])
                return zp

            u0 = load_unit(0)
            zps = [load_zp(0), load_zp(1)]
            zn_t = consts.tile([128, NG, EC, 1024], BF)
            nc.sync.dma_start(zn_t[:, 0], ap["zn"][0])
            addm_t = consts.tile([128, 1024], F32)
            nc.sync.dma_start(addm_t[:], ap["addm"][:])
            eye_t = consts.tile([128, 128], F32)
            nc.sync.dma_start(eye_t[:], ap["eye"][:])
            u1 = load_unit(1)
            for g in range(1, NG):
                nc.sync.dma_start(zn_t[:, g], ap["zn"][g])
            out_t = consts.tile([128, NCHUNKS], F32)
            negM = consts.tile([128, 1], F32)
            nc.vector.memset(negM[:], -M_SHIFT)

            pending = u1
            zh = mm1(*u0)
            for u in range(NUNITS):
                # pipeline: next unit's linear layer + tile loads first so
                # its zh is ready (and ACT casts aren't queued behind this
                # unit's big exp ops)
                zh_next = None
                zps_next = None
                if u + 1 < NUNITS:
                    zps_next = [load_zp(2 * u + 2), load_zp(2 * u + 3)]
                    zh_next = mm1(*pending)
                    if u + 2 < NUNITS:
                        pending = load_unit(u + 2)

                for h_ in range(2):
                    t_idx = 2 * u + h_
                    zp = zps[h_]

                    rs = slice(h_ * 128, (h_ + 1) * 128)

                    # positives: diag(zh_chunk @ Zpos^T)
                    pos_ps = ps_pos.tile([128, 128], F32)
                    for ec in range(EC):
                        nc.tensor.matmul(
                            pos_ps[:], zh[:, ec, rs], zp[:, ec, :],
                            start=(ec == 0), stop=(ec == EC - 1),
                        )

                    masked = mpool.tile([128, 4096], F32)
                    for grp in range(NG):
                        raw_ps = ps_raw.tile([128, 1024], F32)
                        for q in range(2):
                            for ec in range(EC):
                                nc.tensor.matmul(
                                    raw_ps[:, q * 512:(q + 1) * 512],
                                    zh[:, ec, rs],
                                    zn_t[:, grp, ec, q * 512:(q + 1) * 512],
                                    start=(ec == 0),
                                    stop=(ec == EC - 1),
                                )
                        # masked = raw + addmask (one 1024-wide DVE op)
                        nc.vector.tensor_add(
                            masked[:, grp * 1024:(grp + 1) * 1024],
                            raw_ps[:], addm_t[:],
                        )

                    dsc = scr.tile([128, 128], F32)
                    pos_sb = smalls.tile([128, 1], F32)
                    nc.vector.tensor_mul(dsc[:], pos_ps[:], eye_t[:])
                    nc.vector.reduce_sum(out=pos_sb[:], in_=dsc[:], axis=X)

                    # S = sum exp(masked - M), two fused ACT passes so the
                    # first half starts while DVE finishes the second half
                    esc = scr.tile([128, 4096], F32)
                    Sh = smalls.tile([128, 2], F32)
                    for q in range(2):
                        nc.scalar.activation(
                            esc[:, q * 2048:(q + 1) * 2048],
                            masked[:, q * 2048:(q + 1) * 2048], Exp,
                            bias=negM[:, 0:1], scale=1.0,
                            accum_out=Sh[:, q:q + 1],
                        )
                    S = smalls.tile([128, 1], F32)
                    nc.vector.reduce_sum(out=S[:], in_=Sh[:], axis=X)
                    E = smalls.tile([128, 1], F32)
                    nc.scalar.activation(E[:], pos_sb[:], Exp, bias=negM[:, 0:1])
                    T = smalls.tile([128, 1], F32)
                    nc.vector.tensor_add(T[:], E[:], S[:])
                    L = smalls.tile([128, 1], F32)
                    nc.scalar.activation(L[:], T[:], Ln)
                    # nce = (pos - M) - L
                    nc.vector.scalar_tensor_tensor(
                        out=out_t[:, t_idx:t_idx + 1],
                        in0=pos_sb[:],
                        scalar=-M_SHIFT,
                        in1=L[:],
                        op0=Add,
                        op1=Sub,
                    )

                zh = zh_next
                zps = zps_next

            nc.sync.dma_start(out_ap[:], out_t[:])

    _split_multi_waits(nc)
    return nc


def _prep_inputs(Z, C, Wk, bk):
    """Host-side layout prep + per-core slicing (partition-major so every
    SBUF tile loads with a single contiguous DMA)."""
    ii, kk = np.triu_indices(H, 1)
    # (NG, 128, EC, 1024): negatives matrix split into 4 column quarters
    zn = (
        Z.transpose(1, 2, 3, 0).reshape(EC, 128, NG, 1024)
        .transpose(2, 1, 0, 3)
    )
    zn = np.ascontiguousarray(zn).astype(BF16)
    WkT = Wk.transpose(0, 2, 1).reshape(7, EC, 128, 512).transpose(0, 2, 1, 3)
    WkT = np.ascontiguousarray(WkT).astype(BF16)  # (7, 128, 4, 512)
    Ctr = np.ascontiguousarray(C.transpose(2, 1, 3, 0))  # (H, D, W, B)
    Ztr = np.ascontiguousarray(Z.transpose(2, 1, 3, 0))  # (H, D, W, B)

    rr = np.arange(128)
    addm = np.where(
        (np.arange(1024)[None, :] % 64) == (rr[:, None] % 64),
        np.float32(MASK_VAL), np.float32(0.0),
    ).astype(np.float32)
    eye = np.eye(128, dtype=np.float32)

    in_maps = []
    for c in range(NCORES):
        wtc = np.empty((NUNITS, 128, EC, 512), BF16)
        ctc = np.empty((NUNITS, 128, EC, 256), BF16)
        zpc = np.empty((NCHUNKS, 128, EC, 128), BF16)
        bgc = np.empty((NUNITS, 128, EC), np.float32)
        for u in range(NUNITS):
            g = NUNITS * c + u
            p = g // 2
            w0 = 4 * (g % 2)
            i_, k_ = int(ii[p]), int(kk[p])
            wtc[u] = WkT[k_ - 1]
            ctc[u] = (
                Ctr[i_][:, w0:w0 + 4, :].reshape(EC, 128, 256)
                .transpose(1, 0, 2).astype(BF16)
            )
            bgc[u] = bk[k_ - 1].reshape(EC, 128).T
            for h_ in range(2):
                wp0 = w0 + 2 * h_
                zpc[2 * u + h_] = (
                    Ztr[k_][:, wp0:wp0 + 2, :].reshape(EC, 128, 128)
                    .transpose(1, 0, 2).astype(BF16)
                )
        in_maps.append({
            "zn": zn, "wtc": wtc, "ctc": ctc, "zpc": zpc, "bgc": bgc,
            "addm": addm, "eye": eye,
        })
    return in_maps


def kernel(Z, C, Wk, bk):
    global LAST_RESULTS
    Z = np.asarray(Z, np.float32)
    C = np.asarray(C, np.float32)
    Wk = np.asarray(Wk, np.float32)
    bk = np.asarray(bk, np.float32)

    if "nc" not in _cache:
        _cache["nc"] = _build_module()
    nc = _cache["nc"]

    in_maps = _prep_inputs(Z, C, Wk, bk)
    res = run_bass_kernel_spmd(nc, in_maps, core_ids=list(range(NCORES)))
    LAST_RESULTS = res
    total = np.float64(0.0)
    for c in range(NCORES):
        total += np.sum(res.results[c]["out"].astype(np.float64))
    loss = -(total / (NCORES * NCHUNKS * 128))
    return np.array(loss, dtype=np.float32)
